# revision 5
# baseline (speedup 1.0000x reference)
"""Akima spline evaluation (nn_Akima_66623532696299) on 8 Trainium2 cores.

Strategy: data-parallel over the batch axis (8 batches per core). Per element
the spline y(x) is evaluated without any gather via the telescoped identity

    y(x) = v0 + sum_k g_k(d_k),   d_k = clamp(63*x - k, 0, 1)
    g_k(d) = d*(A_k + d*(B_k + d*C_k))

where g_k is segment k's cubic expressed in the normalized local coordinate
(g_k(1) = v_{k+1} - v_k exactly, so partial sums stay O(|v|) and fp32
accumulation is stable). Segment coefficients are derived from `value` on the
host in float64 and baked into the instruction stream as immediates.

Engines: DVE (vector) evaluates every segment's cubic with fused
tensor_scalar / scalar_tensor_tensor ops; the otherwise-idle PE accumulates
the 63 per-segment tiles via identity-weight matmuls into PSUM (native
accumulation, one-way handoff, off the DVE critical path); ACT does the 63*x
prescale and the final PSUM->SBUF drain fused with the +v0 bias; HWDGE
(sync) does the DMA, overlapped via double-buffered tile pools.
"""

import numpy as np

N_CORES = 8
P = 128
B, CH, H, W = 64, 3, 512, 512
PER_CORE = (B // N_CORES) * CH * H * W        # 6291456
FTOT = PER_CORE // P                          # 49152
TF = 2048                                     # tile free size
NT = FTOT // TF                               # 24 tiles
NSEG = 63

_CACHE = {}
LAST_EXEC_NS = None


def _apply_walrus_compat_patches():
    """This container's walrus rejects >1 sync-wait command per instruction;
    Tile's wait assignment can emit several. Split excess waits onto bare
    same-engine NoOps committed immediately before the instruction."""
    import concourse.tile as tile
    from concourse import mybir
    from concourse.vector_clock import ScopedClock

    if getattr(tile.TileContext, "_akima_patched", False):
        return
    MAX_WAITS = 1
    _orig_commit = tile.TileContext._commit_instruction

    def _split_waits(self, inst, lazy_reg_writes=True):
        si = inst.sync_info
        if si is not None and si.on_wait and len(si.on_wait) > MAX_WAITS:
            waits = list(si.on_wait)
            updates = list(si.on_update or [])
            inst.sync_info = mybir.SyncInfo(on_wait=waits[:MAX_WAITS], on_update=updates)
            for i in range(MAX_WAITS, len(waits), MAX_WAITS):
                nop = mybir.InstNoOp(name=f"I-{self.nc.next_id()}", engine=inst.engine)
                nop.sync_info = mybir.SyncInfo(on_wait=waits[i : i + MAX_WAITS], on_update=[])
                _orig_commit(self, nop, lazy_reg_writes)
        return _orig_commit(self, inst, lazy_reg_writes)

    def _drain_and_barrier(self, tick_clock, wait_clock):
        nc = self.nc
        collector = nc.sync.nop(nofuse=True).ins
        wait_clock.add_sem_waits(collector, ScopedClock({None: tick_clock.global_clock}))
        si = collector.sync_info
        waits = list(si.on_wait or []) if si is not None else []
        updates = list(si.on_update or []) if si is not None else []
        if len(waits) > MAX_WAITS:
            collector.sync_info = mybir.SyncInfo(on_wait=waits[:MAX_WAITS], on_update=updates)
            rest = waits[MAX_WAITS:]
            while rest:
                extra = nc.sync.nop(nofuse=True).ins
                extra.sync_info = mybir.SyncInfo(on_wait=rest[:MAX_WAITS], on_update=[])
                rest = rest[MAX_WAITS:]
        nc.sync.drain()
        nc.all_engine_barrier()
        assert self.sems is not None
        popped = nc._tile_sem_poison_stack.pop()
        assert popped is self._sem_poison
        nc.clear_and_free_semaphores(list(self.sems.allocated().values()))
        nc.all_engine_barrier()

    tile.TileContext._commit_instruction = _split_waits
    tile.TileContext._drain_and_barrier = _drain_and_barrier
    tile.TileContext._akima_patched = True


def _coefficients(value):
    """Per-segment cubic coefficients (A, B, C) and v0, in float64, following
    the reference Akima construction."""
    v = np.asarray(value, dtype=np.float64)
    n = v.shape[0]
    h = 1.0 / (n - 1)
    m = np.diff(v) / h
    m_ext = np.concatenate(
        [[3 * m[0] - 2 * m[1], 2 * m[0] - m[1]], m,
         [2 * m[-1] - m[-2], 3 * m[-1] - 2 * m[-2]]]
    )
    dm = np.abs(np.diff(m_ext))
    w1 = dm[2:]
    w2 = dm[:-2]
    den = w1 + w2
    safe = np.where(den > 0, den, 1.0)
    ml = m_ext[1 : n + 1]
    mr = m_ext[2 : n + 2]
    t = np.where(den > 0, (w1 * ml + w2 * mr) / safe, 0.5 * (ml + mr))
    A = (t[:-1] * h).astype(np.float32)
    Bc = ((3 * m - 2 * t[:-1] - t[1:]) * h).astype(np.float32)
    Cc = ((t[:-1] + t[1:] - 2 * m) * h).astype(np.float32)
    return A, Bc, Cc, np.float32(v[0])


def _build_bass(A, Bc, Cc, v0, reps=1):
    import concourse.bass as bass
    import concourse.tile as tile
    from concourse import mybir

    AL = mybir.AluOpType
    AF = mybir.ActivationFunctionType
    F32 = mybir.dt.float32
    nc = bass.Bass()
    x = nc.declare_dram_parameter("x", [P, FTOT], F32, isOutput=False)
    eye = nc.declare_dram_parameter("eye", [P, P], F32, isOutput=False)
    y = nc.declare_dram_parameter("y", [P, FTOT], F32, isOutput=True)
    NCHUNK = TF // 512

    with tile.TileContext(nc) as tc:
        with (
            tc.tile_pool(name="cp", bufs=1) as cp,
            tc.tile_pool(name="xp", bufs=2) as xp,
            tc.tile_pool(name="Xp", bufs=2) as Xp,
            tc.tile_pool(name="up", bufs=3) as up,
            tc.tile_pool(name="pp", bufs=3) as pp,
            tc.tile_pool(name="gp", bufs=4) as gp,
            tc.tile_pool(name="op", bufs=2) as op,
            tc.tile_pool(name="ps", bufs=2, space="PSUM") as ps,
        ):
            eyet = cp.tile([P, P], F32, tag="eye")
            nc.sync.dma_start(eyet[:], eye[:])
            for it in [i % NT for i in range(NT * reps)]:
                xt = xp.tile([P, TF], F32, tag="xt")
                nc.sync.dma_start(xt[:], x[:, bass.ts(it, TF)])
                Xt = Xp.tile([P, TF], F32, tag="Xt")
                nc.scalar.mul(Xt[:], xt[:], 63.0)
                acc = ps.tile([P, TF], F32, tag="acc")
                for k in range(NSEG):
                    a, b, c = float(A[k]), float(Bc[k]), float(Cc[k])
                    u2 = up.tile([P, TF], F32, tag="u2")
                    # u2 = min(X, k+1) - k ; d = max(u2, 0)
                    nc.vector.tensor_scalar(out=u2[:], in0=Xt[:], scalar1=float(k + 1),
                                            scalar2=float(k), op0=AL.min, op1=AL.subtract)
                    nc.vector.tensor_scalar(out=u2[:], in0=u2[:], scalar1=0.0, scalar2=None, op0=AL.max)
                    p = pp.tile([P, TF], F32, tag="p")
                    # p = d*C + B ; q = p*d ; g = (q + A)*d
                    nc.vector.tensor_scalar(out=p[:], in0=u2[:], scalar1=c, scalar2=b, op0=AL.mult, op1=AL.add)
                    nc.vector.tensor_tensor(out=p[:], in0=p[:], in1=u2[:], op=AL.mult)
                    g = gp.tile([P, TF], F32, tag="g")
                    nc.vector.scalar_tensor_tensor(out=g[:], in0=p[:], scalar=a, in1=u2[:], op0=AL.add, op1=AL.mult)
                    # PE: acc (PSUM) += eye.T @ g, one matmul per 512-wide bank
                    for j in range(NCHUNK):
                        nc.tensor.matmul(
                            acc[:, bass.ts(j, 512)], eyet[:], g[:, bass.ts(j, 512)],
                            start=(k == 0), stop=(k == NSEG - 1),
                        )
                yt = op.tile([P, TF], F32, tag="yt")
                nc.scalar.activation(yt[:], acc[:], AF.Copy, bias=float(v0), scale=1.0)
                nc.sync.dma_start(y[:, bass.ts(it, TF)], yt[:])
    return nc


def kernel(input, value):
    global LAST_EXEC_NS
    import time

    _apply_walrus_compat_patches()
    from concourse.bass_utils import run_bass_kernel_spmd

    input = np.ascontiguousarray(np.asarray(input, dtype=np.float32))
    value = np.ascontiguousarray(np.asarray(value, dtype=np.float32))
    key = value.tobytes()
    nc = _CACHE.get(key)
    if nc is None:
        A, Bc, Cc, v0 = _coefficients(value)
        nc = _build_bass(A, Bc, Cc, v0)
        _CACHE.clear()
        _CACHE[key] = nc

    shards = input.reshape(N_CORES, P, FTOT)
    eye = np.eye(P, dtype=np.float32)
    in_maps = [{"x": shards[c], "eye": eye} for c in range(N_CORES)]
    t0 = time.time()
    res = run_bass_kernel_spmd(nc, in_maps, core_ids=list(range(N_CORES)))
    LAST_EXEC_NS = (time.time() - t0) * 1e9
    out = np.stack([res.results[c]["y"] for c in range(N_CORES)], axis=0)
    return out.reshape(B, CH, H, W).astype(np.float32, copy=False)


# revision 6
# speedup vs baseline: 1.2745x; 1.2745x over previous
"""Akima spline evaluation (nn_Akima_66623532696299) on 8 Trainium2 cores.

Strategy: data-parallel over the batch axis (8 batches per core). Per element
the spline y(x) is evaluated without any gather via the telescoped identity

    y(x) = v0 + sum_k g_k(d_k),   d_k = clamp(63*x - k, 0, 1)
    g_k(d) = d*(A_k + d*(B_k + d*C_k))

where g_k is segment k's cubic expressed in the normalized local coordinate
(g_k(1) = v_{k+1} - v_k exactly, so partial sums stay O(|v|) and fp32
accumulation is stable). Segment coefficients are derived from `value` on the
host in float64 and baked into the instruction stream as immediates.

Engines: ACT computes relu(63*x - k) per segment in one fused activation
(scale+bias+relu), feeding one-way into the DVE, which finishes each
segment's cubic in 4 fused ops (min + tensor_scalar + tensor_tensor +
scalar_tensor_tensor); the otherwise-idle PE accumulates the 63 per-segment
tiles via identity-weight matmuls into PSUM (native accumulation, off the
DVE critical path); ACT drains PSUM->SBUF fused with the +v0 bias; HWDGE
(sync) does the DMA. Only one-way producer->consumer engine handoffs are
used - fine-grained round-trips measured slower.
"""

import numpy as np

N_CORES = 8
P = 128
B, CH, H, W = 64, 3, 512, 512
PER_CORE = (B // N_CORES) * CH * H * W        # 6291456
FTOT = PER_CORE // P                          # 49152
TF = 2048                                     # tile free size
NT = FTOT // TF                               # 24 tiles
NSEG = 63

_CACHE = {}
LAST_EXEC_NS = None


def _apply_walrus_compat_patches():
    """This container's walrus rejects >1 sync-wait command per instruction;
    Tile's wait assignment can emit several. Split excess waits onto bare
    same-engine NoOps committed immediately before the instruction."""
    import concourse.tile as tile
    from concourse import mybir
    from concourse.vector_clock import ScopedClock

    if getattr(tile.TileContext, "_akima_patched", False):
        return
    MAX_WAITS = 1
    _orig_commit = tile.TileContext._commit_instruction

    def _split_waits(self, inst, lazy_reg_writes=True):
        si = inst.sync_info
        if si is not None and si.on_wait and len(si.on_wait) > MAX_WAITS:
            waits = list(si.on_wait)
            updates = list(si.on_update or [])
            inst.sync_info = mybir.SyncInfo(on_wait=waits[:MAX_WAITS], on_update=updates)
            for i in range(MAX_WAITS, len(waits), MAX_WAITS):
                nop = mybir.InstNoOp(name=f"I-{self.nc.next_id()}", engine=inst.engine)
                nop.sync_info = mybir.SyncInfo(on_wait=waits[i : i + MAX_WAITS], on_update=[])
                _orig_commit(self, nop, lazy_reg_writes)
        return _orig_commit(self, inst, lazy_reg_writes)

    def _drain_and_barrier(self, tick_clock, wait_clock):
        nc = self.nc
        collector = nc.sync.nop(nofuse=True).ins
        wait_clock.add_sem_waits(collector, ScopedClock({None: tick_clock.global_clock}))
        si = collector.sync_info
        waits = list(si.on_wait or []) if si is not None else []
        updates = list(si.on_update or []) if si is not None else []
        if len(waits) > MAX_WAITS:
            collector.sync_info = mybir.SyncInfo(on_wait=waits[:MAX_WAITS], on_update=updates)
            rest = waits[MAX_WAITS:]
            while rest:
                extra = nc.sync.nop(nofuse=True).ins
                extra.sync_info = mybir.SyncInfo(on_wait=rest[:MAX_WAITS], on_update=[])
                rest = rest[MAX_WAITS:]
        nc.sync.drain()
        nc.all_engine_barrier()
        assert self.sems is not None
        popped = nc._tile_sem_poison_stack.pop()
        assert popped is self._sem_poison
        nc.clear_and_free_semaphores(list(self.sems.allocated().values()))
        nc.all_engine_barrier()

    tile.TileContext._commit_instruction = _split_waits
    tile.TileContext._drain_and_barrier = _drain_and_barrier
    tile.TileContext._akima_patched = True


def _coefficients(value):
    """Per-segment cubic coefficients (A, B, C) and v0, in float64, following
    the reference Akima construction."""
    v = np.asarray(value, dtype=np.float64)
    n = v.shape[0]
    h = 1.0 / (n - 1)
    m = np.diff(v) / h
    m_ext = np.concatenate(
        [[3 * m[0] - 2 * m[1], 2 * m[0] - m[1]], m,
         [2 * m[-1] - m[-2], 3 * m[-1] - 2 * m[-2]]]
    )
    dm = np.abs(np.diff(m_ext))
    w1 = dm[2:]
    w2 = dm[:-2]
    den = w1 + w2
    safe = np.where(den > 0, den, 1.0)
    ml = m_ext[1 : n + 1]
    mr = m_ext[2 : n + 2]
    t = np.where(den > 0, (w1 * ml + w2 * mr) / safe, 0.5 * (ml + mr))
    A = (t[:-1] * h).astype(np.float32)
    Bc = ((3 * m - 2 * t[:-1] - t[1:]) * h).astype(np.float32)
    Cc = ((t[:-1] + t[1:] - 2 * m) * h).astype(np.float32)
    return A, Bc, Cc, np.float32(v[0])


def _build_bass(A, Bc, Cc, v0, reps=1):
    import concourse.bass as bass
    import concourse.tile as tile
    from concourse import mybir

    AL = mybir.AluOpType
    AF = mybir.ActivationFunctionType
    F32 = mybir.dt.float32
    nc = bass.Bass()
    x = nc.declare_dram_parameter("x", [P, FTOT], F32, isOutput=False)
    eye = nc.declare_dram_parameter("eye", [P, P], F32, isOutput=False)
    kb = nc.declare_dram_parameter("kb", [P, NSEG], F32, isOutput=False)
    y = nc.declare_dram_parameter("y", [P, FTOT], F32, isOutput=True)
    NCHUNK = TF // 512

    with tile.TileContext(nc) as tc:
        with (
            tc.tile_pool(name="cp", bufs=1) as cp,
            tc.tile_pool(name="xp", bufs=2) as xp,
            tc.tile_pool(name="rp", bufs=3) as rp,
            tc.tile_pool(name="pp", bufs=3) as pp,
            tc.tile_pool(name="gp", bufs=4) as gp,
            tc.tile_pool(name="op", bufs=2) as op,
            tc.tile_pool(name="ps", bufs=2, space="PSUM") as ps,
        ):
            eyet = cp.tile([P, P], F32, tag="eye")
            nc.sync.dma_start(eyet[:], eye[:])
            kbt = cp.tile([P, NSEG], F32, tag="kbt")
            nc.sync.dma_start(kbt[:], kb[:])
            for it in [i % NT for i in range(NT * reps)]:
                xt = xp.tile([P, TF], F32, tag="xt")
                nc.sync.dma_start(xt[:], x[:, bass.ts(it, TF)])
                acc = ps.tile([P, TF], F32, tag="acc")
                for k in range(NSEG):
                    a, b, c = float(A[k]), float(Bc[k]), float(Cc[k])
                    r = rp.tile([P, TF], F32, tag="r")
                    # ACT: r = relu(63*x - k); DVE: d = min(r, 1)
                    nc.scalar.activation(r[:], xt[:], AF.Relu, bias=kbt[:, k : k + 1], scale=63.0)
                    nc.vector.tensor_scalar(out=r[:], in0=r[:], scalar1=1.0, scalar2=None, op0=AL.min)
                    p = pp.tile([P, TF], F32, tag="p")
                    # p = d*C + B ; q = p*d ; g = (q + A)*d
                    nc.vector.tensor_scalar(out=p[:], in0=r[:], scalar1=c, scalar2=b, op0=AL.mult, op1=AL.add)
                    nc.vector.tensor_tensor(out=p[:], in0=p[:], in1=r[:], op=AL.mult)
                    g = gp.tile([P, TF], F32, tag="g")
                    nc.vector.scalar_tensor_tensor(out=g[:], in0=p[:], scalar=a, in1=r[:], op0=AL.add, op1=AL.mult)
                    # PE: acc (PSUM) += eye.T @ g, one matmul per 512-wide bank
                    for j in range(NCHUNK):
                        nc.tensor.matmul(
                            acc[:, bass.ts(j, 512)], eyet[:], g[:, bass.ts(j, 512)],
                            start=(k == 0), stop=(k == NSEG - 1),
                        )
                yt = op.tile([P, TF], F32, tag="yt")
                nc.scalar.activation(yt[:], acc[:], AF.Copy, bias=float(v0), scale=1.0)
                nc.sync.dma_start(y[:, bass.ts(it, TF)], yt[:])
    return nc


def kernel(input, value):
    global LAST_EXEC_NS
    import time

    _apply_walrus_compat_patches()
    from concourse.bass_utils import run_bass_kernel_spmd

    input = np.ascontiguousarray(np.asarray(input, dtype=np.float32))
    value = np.ascontiguousarray(np.asarray(value, dtype=np.float32))
    key = value.tobytes()
    nc = _CACHE.get(key)
    if nc is None:
        A, Bc, Cc, v0 = _coefficients(value)
        nc = _build_bass(A, Bc, Cc, v0)
        _CACHE.clear()
        _CACHE[key] = nc

    shards = input.reshape(N_CORES, P, FTOT)
    eye = np.eye(P, dtype=np.float32)
    kb = np.broadcast_to(-np.arange(NSEG, dtype=np.float32), (P, NSEG)).copy()
    in_maps = [{"x": shards[c], "eye": eye, "kb": kb} for c in range(N_CORES)]
    t0 = time.time()
    res = run_bass_kernel_spmd(nc, in_maps, core_ids=list(range(N_CORES)))
    LAST_EXEC_NS = (time.time() - t0) * 1e9
    out = np.stack([res.results[c]["y"] for c in range(N_CORES)], axis=0)
    return out.reshape(B, CH, H, W).astype(np.float32, copy=False)
